# revision 1
# baseline (speedup 1.0000x reference)
"""Multi-head attention forward (B=8, S=1024, H=16, D=64) on 8 TRN2 NeuronCores.

Sharding: pure data-parallel over batch — core b computes batch element b
end-to-end (QKV projections + 16-head attention). Zero collectives.

Per-core dataflow (bf16 matmuls, fp32 PSUM accumulation):
  phase 0: x loads on the HWDGE queue (f32) + DVE cast to bf16 +
           PE-transpose to x^T layout; weight loads (cast to bf16 in
           SWDGE) run on the gpsimd queue in parallel.
  pair loop (8 head-pairs, interleaved so the PE always has dense work and
  ScalarE's exp stream starts as early as possible):
    - Q^T/K^T/V^T slices for this pair (lhsT = weight slice, rhs = x^T,
      N=512 moving, bias via per-partition tensor_scalar on the way out
      of PSUM; V bias is exact here: softmax rows sum to 1, so
      normalize(P_u @ (V+bv)) == ctx + bv)
    - V' strips [V_h | ones] per s-tile via PE-transpose of V^T (ones
      column -> softmax denominator lands in the ctx matmul for free)
    - scores^T[j,i] = K_h^T.T @ Q_h^T (K=64 contraction; the two heads of
      a pair sit at SBUF partitions 0-63/64-127 so their matmuls land on
      disjoint PE row-groups and run concurrently)
    - Et = exp(scores^T/8) on ScalarE (no max-subtraction: logits bounded
      ~|2.3| for these inputs)
    - ctx'^T[65,i] = sum_jt V'_jt.T @ Et_jt (row 64 = softmax denominator)
    - PE-transpose ctx' back to [i,d], multiply by the reciprocal of the
      denominator column, DMA this pair's 128 output columns out.
"""

import numpy as np
from contextlib import ExitStack

import concourse.bass as bass
import concourse.mybir as mybir
import concourse.tile as tile
from concourse import bacc
from concourse.masks import make_identity
from concourse.bass_utils import run_bass_kernel_spmd

B, S, H, D = 8, 1024, 16, 64
W = H * D  # 1024
P = 128
N_CORES = 8
F32 = mybir.dt.float32
BF16 = mybir.dt.bfloat16
AF = mybir.ActivationFunctionType
ALU = mybir.AluOpType

ST = S // P   # 8 s-tiles
KT_ = W // P  # 8 contraction tiles
IH = 2        # 512-wide halves of the moving dim
HD1 = D + 1   # 65: V' width per head
NP = H // 2   # 8 head pairs


def _dedup_ldweights(nc):
    """Drop InstLdweights that reload the exact weights already resident in
    the PE array (the two ih-halves of each projection chain step share one
    stationary). Runs post-compile, so syncs are final: only duplicates with
    empty sync_info, separated from the previous load purely by matmuls on
    the PE stream, are removed — the weights are untouched in the array and
    the instruction is a pure re-load."""
    removed = 0
    for f in nc.m.functions:
        for blk in f.blocks:
            ins = blk.instructions
            last_key = None
            to_remove = []
            for i in ins:
                if str(getattr(i, "engine", None)) != "EngineType.PE":
                    continue
                tn = type(i).__name__
                if tn == "InstLdweights":
                    si = i.sync_info
                    clean = si is None or (not si.on_wait and not si.on_update)
                    key = (str(i.ins), str(getattr(i, "is_transpose", None)),
                           str(getattr(i, "tile_position", None)),
                           str(getattr(i, "perf_mode", None)))
                    if clean and key == last_key:
                        to_remove.append(i)
                    else:
                        last_key = key
                elif tn != "InstMatmult":
                    # anything else on PE: conservatively forget the residency
                    last_key = None
            for i in to_remove:
                ins.remove(i)
            removed += len(to_remove)
    return removed


def build_kernel():
    nc = bacc.Bacc(trn_type="TRN2", target_bir_lowering=False, debug=False,
                   num_devices=N_CORES)

    xf_ext = nc.dram_tensor("from_tensor", [S, W], F32, kind="ExternalInput").ap()
    xt_ext = nc.dram_tensor("to_tensor", [S, W], F32, kind="ExternalInput").ap()
    wq_ext = nc.dram_tensor("Wq", [W, W], F32, kind="ExternalInput").ap()
    bq_ext = nc.dram_tensor("bq", [W], F32, kind="ExternalInput").ap()
    wk_ext = nc.dram_tensor("Wk", [W, W], F32, kind="ExternalInput").ap()
    bk_ext = nc.dram_tensor("bk", [W], F32, kind="ExternalInput").ap()
    wv_ext = nc.dram_tensor("Wv", [W, W], F32, kind="ExternalInput").ap()
    bv_ext = nc.dram_tensor("bv", [W], F32, kind="ExternalInput").ap()
    out_ext = nc.dram_tensor("out", [S, W], F32, kind="ExternalOutput").ap()

    with tile.TileContext(nc) as tc, ExitStack() as top:
        const = top.enter_context(tc.tile_pool(name="const", bufs=1))
        big = top.enter_context(tc.tile_pool(name="big", bufs=1))

        ident = const.tile([P, P], BF16, tag="ident")
        make_identity(nc, ident[:])
        # biases ride the gpsimd (SWDGE) queue so the sync queue starts with
        # the x chunks the first PE transposes are waiting on
        bq_sb = const.tile([P, KT_], F32, tag="bq")
        nc.gpsimd.dma_start(bq_sb[:], bq_ext.rearrange("(t p) -> p t", p=P))
        bk_sb = const.tile([P, KT_], F32, tag="bk")
        nc.gpsimd.dma_start(bk_sb[:], bk_ext.rearrange("(t p) -> p t", p=P))
        bv_sb = const.tile([P, KT_], F32, tag="bv")
        nc.gpsimd.dma_start(bv_sb[:], bv_ext.rearrange("(t p) -> p t", p=P))

        # xT_all[p, kt*S + s] = x[s, kt*128+p]
        xTf_all = big.tile([P, KT_ * S], BF16, tag="xTf")
        xTt_all = big.tile([P, KT_ * S], BF16, tag="xTt")
        # w_all[p, kt*W + f] = Wx[kt*128+p, f]
        wq_all = big.tile([P, KT_ * W], BF16, tag="wq")
        wk_all = big.tile([P, KT_ * W], BF16, tag="wk")
        wv_all = big.tile([P, KT_ * W], BF16, tag="wv")

        def load_w(dst, src):
            nc.gpsimd.dma_start(
                dst.rearrange("p (t f) -> p t f", f=W),
                src.rearrange("(t p) f -> p t f", p=P))

        # ---- phase 0: load + cast + transpose inputs ----
        with ExitStack() as ph0:
            xr_pool = ph0.enter_context(tc.tile_pool(name="xr", bufs=2))
            xf_pool = ph0.enter_context(tc.tile_pool(name="xf", bufs=2))
            ps_t = ph0.enter_context(
                tc.tile_pool(name="ps_t", bufs=4, space="PSUM"))

            def transpose_chunk(x_ext, xT_all, ch):
                xr = xr_pool.tile([P, 2 * W], F32, tag="xr", name=f"xr{ch}")
                nc.sync.dma_start(
                    xr.rearrange("p (t f) -> p t f", f=W),
                    x_ext.rearrange("(t p) f -> p t f", p=P)[
                        :, ch * 2:(ch + 1) * 2, :])
                xf = xf_pool.tile([P, 2 * W], BF16, tag="xf", name=f"xf{ch}")
                nc.vector.tensor_copy(xf[:], xr[:])
                for wt in range(KT_):
                    pt = ps_t.tile([P, 256], BF16, tag="pt", bufs=4, name="pt")
                    for sl in range(2):
                        nc.tensor.transpose(
                            pt[:, sl * P:(sl + 1) * P],
                            xf[:, sl * W + wt * P: sl * W + wt * P + P],
                            ident[:])
                    nc.vector.tensor_copy(
                        xT_all[:, wt * S + ch * 256: wt * S + (ch + 1) * 256],
                        pt[:])

            # x_from streams in completely before x_to: with the HBM-in
            # saturated by the parallel weight loads, chunk interleaving
            # would delay x_from's completion (and with it pair-0's Q
            # projection and the whole ScalarE exp stream) by ~15us
            for ch in range(4):
                transpose_chunk(xf_ext, xTf_all, ch)
                if ch == 0:
                    load_w(wq_all, wq_ext)
                    load_w(wk_all, wk_ext)
            for ch in range(4):
                transpose_chunk(xt_ext, xTt_all, ch)
            load_w(wv_all, wv_ext)

        # ---- pair loop ----
        with ExitStack() as ph2:
            pp_pool = ph2.enter_context(tc.tile_pool(name="pp", bufs=1))
            et_pool = ph2.enter_context(tc.tile_pool(name="et", bufs=18))
            sm_pool = ph2.enter_context(tc.tile_pool(name="sm", bufs=1))
            ps_proj = ph2.enter_context(
                tc.tile_pool(name="ps_proj", bufs=2, space="PSUM"))
            ps_s = ph2.enter_context(
                tc.tile_pool(name="ps_s", bufs=1, space="PSUM"))
            ps_c = ph2.enter_context(
                tc.tile_pool(name="ps_c", bufs=1, space="PSUM"))

            def proj_pair(dstT, w_all, xT_all, b_sb, mt):
                for ih in range(IH):
                    ps = ps_proj.tile([P, 512], F32, tag="proj", name="pp")
                    for kt in range(KT_):
                        nc.tensor.matmul(
                            ps[:],
                            lhsT=w_all[:, kt * W + mt * P: kt * W + mt * P + P],
                            rhs=xT_all[:, kt * S + ih * 512:
                                       kt * S + (ih + 1) * 512],
                            start=(kt == 0), stop=(kt == KT_ - 1))
                    nc.vector.tensor_scalar_add(
                        dstT[:, ih * 512:(ih + 1) * 512], ps[:],
                        b_sb[:, mt:mt + 1])

            def emit_front(hp):
                """Q/K projections + scores/exp for pair hp."""
                mt = hp  # w-tile index of this pair's 128 output columns
                QTp = pp_pool.tile([P, S], BF16, tag="qt", bufs=2, name="QTp")
                KTp = pp_pool.tile([P, S], BF16, tag="kt", bufs=2, name="KTp")
                proj_pair(QTp, wq_all, xTf_all, bq_sb, mt)
                proj_pair(KTp, wk_all, xTt_all, bk_sb, mt)

                # scores^T + exp; both heads of the pair share ONE 4-bank
                # PSUM tile so their K=64 matmuls are always adjacent in the
                # PE stream — consecutive ops hit disjoint row-groups
                # (0-63 / 64-127) and disjoint banks, packing concurrently
                # in the array. One FD=2048 exp covers both heads.
                Et = {}
                for jt in range(ST):
                    pss = ps_s.tile([P, 2 * S], F32, tag="pss", name="pss")
                    for ih in range(IH):
                        for hh in range(2):
                            ho = hh * D
                            nc.tensor.matmul(
                                pss[:, hh * S + ih * 512:
                                    hh * S + (ih + 1) * 512],
                                lhsT=KTp[ho:ho + D, jt * P: jt * P + P],
                                rhs=QTp[ho:ho + D, ih * 512:(ih + 1) * 512],
                                start=True, stop=True)
                    et = et_pool.tile([P, 2 * S], BF16, tag="et", name="et")
                    nc.scalar.activation(et[:], pss[:], AF.Exp, scale=0.125)
                    Et[jt] = et
                return Et

            def emit_vprime(hp):
                """V projection + V' strips for pair hp (only needed by the
                back half, so emitted after the scores/exp front)."""
                mt = hp
                VTp = pp_pool.tile([P, S], BF16, tag="vt", bufs=2, name="VTp")
                proj_pair(VTp, wv_all, xTt_all, bv_sb, mt)
                Vp = pp_pool.tile([P, ST * 2 * HD1], BF16, tag="vp", bufs=2,
                                  name="Vp")
                for jt in range(ST):
                    for hh in range(2):
                        pv = ps_proj.tile([P, D], BF16, tag="proj", name="pv")
                        ho = hh * D
                        nc.tensor.transpose(
                            pv[:], VTp[ho:ho + D, jt * P:(jt + 1) * P],
                            ident[ho:ho + D, ho:ho + D])
                        nc.vector.tensor_copy(
                            Vp[:, (jt * 2 + hh) * HD1: (jt * 2 + hh) * HD1 + D],
                            pv[:])
                    nc.vector.memset(
                        Vp[:, jt * 2 * HD1: (jt + 1) * 2 * HD1].rearrange(
                            "p (g c) -> p g c", c=HD1)[:, :, D:HD1], 1.0)
                return Vp

            def emit_back(hp, Vp, Et):
                """ctx' + normalize + transpose-out + DMA for pair hp."""
                mt = hp
                out_p = pp_pool.tile([P, ST * P], F32, tag="outp", bufs=2,
                                     name="out_p")
                for hh in range(2):
                    pc = ps_c.tile([HD1, S], F32, tag="pcc", name="pcc")
                    for ih in range(IH):
                        for jt in range(ST):
                            nc.tensor.matmul(
                                pc[:, ih * 512:(ih + 1) * 512],
                                lhsT=Vp[:, (jt * 2 + hh) * HD1:
                                        (jt * 2 + hh + 1) * HD1],
                                rhs=Et[jt][:, hh * S + ih * 512:
                                            hh * S + (ih + 1) * 512],
                                start=(jt == 0), stop=(jt == ST - 1))
                    ctxb = sm_pool.tile([HD1, S], BF16, tag="ctxb", bufs=3,
                                        name="ctxb")
                    nc.vector.tensor_copy(ctxb[:], pc[:])
                    for it in range(ST):
                        po = ps_proj.tile([P, HD1], BF16, tag="proj", name="po")
                        nc.tensor.transpose(
                            po[:], ctxb[:, it * P:(it + 1) * P],
                            ident[0:HD1, 0:HD1])
                        rinv = sm_pool.tile([P, 1], F32, tag="rinv", bufs=4,
                                            name="rinv")
                        nc.vector.reciprocal(rinv[:], po[:, D:HD1])
                        nc.vector.tensor_scalar_mul(
                            out_p[:, it * P + hh * D: it * P + hh * D + D],
                            po[:, 0:D], rinv[:])

                nc.sync.dma_start(
                    out_ext.rearrange("(t p) (g c) -> p t g c", p=P, c=P)[
                        :, :, mt, :],
                    out_p.rearrange("p (t c) -> p t c", c=P))

            # software pipeline: the back half of pair p is emitted after the
            # scores/exp front of pair p+1, so the PE always has ready work
            # queued while ScalarE streams through pair p+1's exps.
            pending = None
            for hp in range(NP):
                Et = emit_front(hp)
                Vp = emit_vprime(hp)
                if pending is not None:
                    emit_back(hp - 1, *pending)
                pending = (Vp, Et)
            emit_back(NP - 1, *pending)

    nc.compile()
    return nc


def run(inputs, trace=False, trace_kwargs=None):
    """inputs: dict of full-shape np arrays as in reference.setup_inputs()."""
    nc = build_kernel()
    in_maps = []
    for b in range(N_CORES):
        in_maps.append({
            "from_tensor": np.ascontiguousarray(np.asarray(inputs["from_tensor"][b], dtype=np.float32)),
            "to_tensor": np.ascontiguousarray(np.asarray(inputs["to_tensor"][b], dtype=np.float32)),
            "Wq": np.asarray(inputs["Wq"], dtype=np.float32),
            "bq": np.asarray(inputs["bq"], dtype=np.float32),
            "Wk": np.asarray(inputs["Wk"], dtype=np.float32),
            "bk": np.asarray(inputs["bk"], dtype=np.float32),
            "Wv": np.asarray(inputs["Wv"], dtype=np.float32),
            "bv": np.asarray(inputs["bv"], dtype=np.float32),
        })
    res = run_bass_kernel_spmd(nc, in_maps, core_ids=list(range(N_CORES)),
                               trace=trace, **(trace_kwargs or {}))
    out = np.stack([np.asarray(res.results[b]["out"]) for b in range(N_CORES)],
                   axis=0).astype(np.float32)
    return out, res


def kernel(**inputs):
    out, _ = run(inputs, trace=False)
    return out



# revision 4
# speedup vs baseline: 1.0432x; 1.0432x over previous
"""Multi-head attention forward (B=8, S=1024, H=16, D=64) on 8 TRN2 NeuronCores.

Sharding: pure data-parallel over batch — core b computes batch element b
end-to-end (QKV projections + 16-head attention). Zero collectives.

Per-core dataflow (bf16 matmuls, fp32 PSUM accumulation), restructured for
PE continuity:
  - biases load as [8,128] rows (8 fat descriptors) + one PE transpose each,
    keeping the SWDGE queue free for weight streaming from t~1us.
  - weights load in column halves (wq/wk/wv lo then hi) so pair 0's slices
    land first; x loads in 256-row chunks on the HWDGE queue.
  - x-chunk transposes are interleaved with pair-0's Q/K projection chains so
    the PE starts real work as soon as half of x has landed.
  - pair loop software-pipelines back(p-1) work (ctx, out-transpose,
    normalize, store) into the gaps of front(p) (projections, scores, exp)
    at ~1.7us granularity so the PE never drains while ScalarE streams exps.
  - V' strips are built with one [128,128] PE transpose per s-tile (both
    heads at once); softmax denominators ride the ones-column of V' and are
    reciprocal'd 8-at-a-time after the output transpose.
"""

import numpy as np
from contextlib import ExitStack

import concourse.bass as bass
import concourse.mybir as mybir
import concourse.tile as tile
from concourse import bacc
from concourse.masks import make_identity
from concourse.bass_utils import run_bass_kernel_spmd

B, S, H, D = 8, 1024, 16, 64
W = H * D  # 1024
P = 128
N_CORES = 8
F32 = mybir.dt.float32
BF16 = mybir.dt.bfloat16
AF = mybir.ActivationFunctionType
ALU = mybir.AluOpType

ST = S // P   # 8 s-tiles
KT_ = W // P  # 8 contraction tiles
IH = 2        # 512-wide halves of the moving dim
HD1 = D + 1   # 65: V' width per head
NP = H // 2   # 8 head pairs
VW = 2 * HD1  # 130: V' slot width per s-tile (two heads + ones cols)


def _dedup_ldweights(nc):
    """Drop InstLdweights that reload the exact weights already resident in
    the PE array. Runs post-compile, so syncs are final: only duplicates with
    empty sync_info, separated from the previous load purely by matmuls on
    the PE stream, are removed."""
    removed = 0
    for f in nc.m.functions:
        for blk in f.blocks:
            ins = blk.instructions
            last_key = None
            to_remove = []
            for i in ins:
                if str(getattr(i, "engine", None)) != "EngineType.PE":
                    continue
                tn = type(i).__name__
                if tn == "InstLdweights":
                    si = i.sync_info
                    clean = si is None or (not si.on_wait and not si.on_update)
                    key = (str(i.ins), str(getattr(i, "is_transpose", None)),
                           str(getattr(i, "tile_position", None)),
                           str(getattr(i, "perf_mode", None)))
                    if clean and key == last_key:
                        to_remove.append(i)
                    else:
                        last_key = key
                elif tn != "InstMatmult":
                    last_key = None
            for i in to_remove:
                ins.remove(i)
            removed += len(to_remove)
    return removed


def build_kernel():
    nc = bacc.Bacc(trn_type="TRN2", target_bir_lowering=False, debug=False,
                   num_devices=N_CORES)

    xf_ext = nc.dram_tensor("from_tensor", [S, W], F32, kind="ExternalInput").ap()
    xt_ext = nc.dram_tensor("to_tensor", [S, W], F32, kind="ExternalInput").ap()
    wq_ext = nc.dram_tensor("Wq", [W, W], F32, kind="ExternalInput").ap()
    bq_ext = nc.dram_tensor("bq", [W], F32, kind="ExternalInput").ap()
    wk_ext = nc.dram_tensor("Wk", [W, W], F32, kind="ExternalInput").ap()
    bk_ext = nc.dram_tensor("bk", [W], F32, kind="ExternalInput").ap()
    wv_ext = nc.dram_tensor("Wv", [W, W], F32, kind="ExternalInput").ap()
    bv_ext = nc.dram_tensor("bv", [W], F32, kind="ExternalInput").ap()
    out_ext = nc.dram_tensor("out", [S, W], F32, kind="ExternalOutput").ap()

    with tile.TileContext(nc) as tc, ExitStack() as top:
        const = top.enter_context(tc.tile_pool(name="const", bufs=1))
        big = top.enter_context(tc.tile_pool(name="big", bufs=1))
        xr_pool = top.enter_context(tc.tile_pool(name="xr", bufs=2))
        xc_pool = top.enter_context(tc.tile_pool(name="xc", bufs=2))
        pp_pool = top.enter_context(tc.tile_pool(name="pp", bufs=1))
        et_pool = top.enter_context(tc.tile_pool(name="et", bufs=17))
        sm_pool = top.enter_context(tc.tile_pool(name="sm", bufs=1))
        ps_proj = top.enter_context(
            tc.tile_pool(name="ps_proj", bufs=2, space="PSUM"))
        ps_s = top.enter_context(
            tc.tile_pool(name="ps_s", bufs=1, space="PSUM"))
        ps_c = top.enter_context(
            tc.tile_pool(name="ps_c", bufs=2, space="PSUM"))

        # ---- identity matrices (gpsimd queue, before weight descriptors) ----
        ident = const.tile([P, P], BF16, tag="ident")
        make_identity(nc, ident[:])
        idf32 = const.tile([8, 8], F32, tag="idf32")
        make_identity(nc, idf32[:])

        # ---- DMA issue: biases + x chunks on sync (HWDGE) queue ----
        brow = const.tile([8, 3 * P], F32, tag="brow")
        for i, b_ext in enumerate((bq_ext, bk_ext, bv_ext)):
            nc.sync.dma_start(brow[:, i * P:(i + 1) * P],
                              b_ext.rearrange("(t p) -> t p", p=P))

        # x chunk tiles: 4 chunks of 256 rows per tensor; dma_starts emitted
        # up-front (ring WAR pacing keeps them flowing as transposes finish)
        xTf_all = big.tile([P, KT_ * S], BF16, tag="xTf")
        xTt_all = big.tile([P, KT_ * S], BF16, tag="xTt")

        def x_chunk_load(x_ext, ch, name):
            xr = xr_pool.tile([P, 2 * W], F32, tag="xr", name=name)
            nc.sync.dma_start(
                xr.rearrange("p (t f) -> p t f", f=W),
                x_ext.rearrange("(t p) f -> p t f", p=P)[
                    :, ch * 2:(ch + 1) * 2, :])
            return xr

        # ---- weight loads in column halves (gpsimd / SWDGE queue) ----
        wq_all = big.tile([P, KT_ * W], BF16, tag="wq")
        wk_all = big.tile([P, KT_ * W], BF16, tag="wk")
        wv_all = big.tile([P, KT_ * W], BF16, tag="wv")

        def load_w_half(dst, src, h):
            nc.gpsimd.dma_start(
                dst.rearrange("p (t f) -> p t f", f=W)[
                    :, :, h * 512:(h + 1) * 512],
                src.rearrange("(t p) f -> p t f", p=P)[
                    :, :, h * 512:(h + 1) * 512])

        load_w_half(wq_all, wq_ext, 0)
        load_w_half(wk_all, wk_ext, 0)
        load_w_half(wv_all, wv_ext, 0)
        load_w_half(wq_all, wq_ext, 1)
        load_w_half(wk_all, wk_ext, 1)
        load_w_half(wv_all, wv_ext, 1)

        # ---- bias transpose: [8,128] rows -> [128,8] columns ----
        b_sb = const.tile([P, 24], F32, tag="b_sb")
        bps = ps_proj.tile([P, 24], F32, tag="proj", name="bps")
        for i in range(3):
            nc.tensor.transpose(bps[:, i * 8:(i + 1) * 8],
                                brow[:, i * P:(i + 1) * P], idf32[:])
        nc.vector.tensor_copy(b_sb[:], bps[:])
        bq_sb = b_sb[:, 0:8]
        bk_sb = b_sb[:, 8:16]
        bv_sb = b_sb[:, 16:24]

        # ---- x chunk processing: cast + 16 transposes + 2 batched copies ----
        def x_chunk_process(xr, xT_all, ch, name):
            xc = xc_pool.tile([P, 2 * W], BF16, tag="xc", name=name)
            nc.vector.tensor_copy(xc[:], xr[:])
            for sl in range(2):
                pt = ps_proj.tile([P, KT_ * P], BF16, tag="proj", name="ptx")
                for wt in range(KT_):
                    nc.tensor.transpose(
                        pt[:, wt * P:(wt + 1) * P],
                        xc[:, sl * W + wt * P: sl * W + wt * P + P],
                        ident[:])
                nc.vector.tensor_copy(
                    xT_all.rearrange("p (w s) -> p w s", s=S)[
                        :, :, ch * 256 + sl * P: ch * 256 + (sl + 1) * P],
                    pt.rearrange("p (w c) -> p w c", c=P))

        # ---- pair-loop building blocks ----
        def proj_half(dstT, w_all, xT_all, b_sl, mt, ih):
            ps = ps_proj.tile([P, 512], F32, tag="proj", name="pp")
            for kt in range(KT_):
                nc.tensor.matmul(
                    ps[:],
                    lhsT=w_all[:, kt * W + mt * P: kt * W + mt * P + P],
                    rhs=xT_all[:, kt * S + ih * 512: kt * S + (ih + 1) * 512],
                    start=(kt == 0), stop=(kt == KT_ - 1))
            nc.vector.tensor_scalar_add(
                dstT[:, ih * 512:(ih + 1) * 512], ps[:], b_sl[:, mt:mt + 1])

        def scores_jt(QTp, KTp, jt, Et):
            # both heads of the pair share ONE 4-bank PSUM tile; their K=64
            # matmuls are adjacent in the PE stream and pack onto disjoint
            # row-groups (0-63 / 64-127), running concurrently.
            pss = ps_s.tile([P, 2 * S], F32, tag="pss", name="pss")
            for ih in range(IH):
                for hh in range(2):
                    ho = hh * D
                    nc.tensor.matmul(
                        pss[:, hh * S + ih * 512: hh * S + (ih + 1) * 512],
                        lhsT=KTp[ho:ho + D, jt * P: jt * P + P],
                        rhs=QTp[ho:ho + D, ih * 512:(ih + 1) * 512],
                        start=True, stop=True)
            et = et_pool.tile([P, 2 * S], BF16, tag="et", name="et")
            nc.scalar.activation(et[:], pss[:], AF.Exp, scale=0.125)
            Et[jt] = et

        def vprime_strips(VTp, Vp, jts):
            # one [128,128] transpose per s-tile covers both heads' V strips
            for jt in jts:
                pv = ps_proj.tile([P, P], BF16, tag="proj", name="pv")
                nc.tensor.transpose(pv[:], VTp[:, jt * P:(jt + 1) * P],
                                    ident[:])
                nc.vector.tensor_copy(
                    Vp.rearrange("p (j g c) -> p j g c", g=2, c=HD1)[
                        :, jt, :, 0:D],
                    pv.rearrange("p (g c) -> p g c", c=D))

        def vprime_ones(Vp):
            nc.vector.memset(
                Vp.rearrange("p (j g c) -> p j g c", g=2, c=HD1)[
                    :, :, :, D:HD1], 1.0)

        def ctx_chunk(prev, hh, ih):
            Vp, Et = prev["Vp"], prev["Et"]
            pc = ps_c.tile([HD1, 512], F32, tag="pc", name="pc")
            for jt in range(ST):
                nc.tensor.matmul(
                    pc[:],
                    lhsT=Vp[:, jt * VW + hh * HD1: jt * VW + (hh + 1) * HD1],
                    rhs=Et[jt][:, hh * S + ih * 512: hh * S + (ih + 1) * 512],
                    start=(jt == 0), stop=(jt == ST - 1))
            nc.vector.tensor_copy(
                prev["ctxb"][hh][:, ih * 512:(ih + 1) * 512], pc[:])

        def outT_norm(prev, hh):
            ctxb = prev["ctxb"][hh]
            out_p = prev["out_p"]
            # stride 66 keeps each bf16 PSUM slice 4-byte aligned
            po = ps_proj.tile([P, ST * 66], BF16, tag="proj", name="po")
            for it in range(ST):
                nc.tensor.transpose(
                    po[:, it * 66: it * 66 + HD1],
                    ctxb[:, it * P:(it + 1) * P],
                    ident[0:HD1, 0:HD1])
            rinv = sm_pool.tile([P, 8], F32, tag="rinv", bufs=3, name="rinv")
            nc.vector.reciprocal(
                rinv.rearrange("p (a b) -> p a b", b=1),
                po.rearrange("p (it c) -> p it c", c=66)[:, :, D:HD1])
            for it in range(ST):
                nc.vector.tensor_scalar_mul(
                    out_p[:, it * P + hh * D: it * P + hh * D + D],
                    po[:, it * 66: it * 66 + D], rinv[:, it:it + 1])

        def out_dma(prev):
            nc.sync.dma_start(
                out_ext.rearrange("(t p) (g c) -> p t g c", p=P, c=P)[
                    :, :, prev["mt"], :],
                prev["out_p"].rearrange("p (t c) -> p t c", c=P))

        def new_back_state(mt):
            ctxb0 = sm_pool.tile([HD1, S], BF16, tag="ctxb", bufs=3,
                                 name="ctxb0")
            ctxb1 = sm_pool.tile([HD1, S], BF16, tag="ctxb", bufs=3,
                                 name="ctxb1")
            out_p = pp_pool.tile([P, ST * P], F32, tag="outp", bufs=2,
                                 name="out_p")
            return {"mt": mt, "ctxb": (ctxb0, ctxb1), "out_p": out_p}

        # ---- pair 0 + phase 0, interleaved ----
        xf_chunks = [x_chunk_load(xf_ext, ch, f"xrf{ch}") for ch in range(2)]
        Et0 = {}
        QT0 = pp_pool.tile([P, S], BF16, tag="qt", bufs=2, name="QT0")
        KT0 = pp_pool.tile([P, S], BF16, tag="kt", bufs=2, name="KT0")
        VT0 = pp_pool.tile([P, S], BF16, tag="vt", bufs=2, name="VT0")
        Vp0 = pp_pool.tile([P, ST * VW], BF16, tag="vp", bufs=2, name="Vp0")

        x_chunk_process(xf_chunks[0], xTf_all, 0, "xcf0")
        xf_chunks.append(x_chunk_load(xf_ext, 2, "xrf2"))
        x_chunk_process(xf_chunks[1], xTf_all, 1, "xcf1")
        xf_chunks.append(x_chunk_load(xf_ext, 3, "xrf3"))
        proj_half(QT0, wq_all, xTf_all, bq_sb, 0, 0)
        x_chunk_process(xf_chunks[2], xTf_all, 2, "xcf2")
        xt_chunks = [x_chunk_load(xt_ext, 0, "xrt0")]
        x_chunk_process(xf_chunks[3], xTf_all, 3, "xcf3")
        xt_chunks.append(x_chunk_load(xt_ext, 1, "xrt1"))
        proj_half(QT0, wq_all, xTf_all, bq_sb, 0, 1)
        x_chunk_process(xt_chunks[0], xTt_all, 0, "xct0")
        xt_chunks.append(x_chunk_load(xt_ext, 2, "xrt2"))
        x_chunk_process(xt_chunks[1], xTt_all, 1, "xct1")
        xt_chunks.append(x_chunk_load(xt_ext, 3, "xrt3"))
        proj_half(KT0, wk_all, xTt_all, bk_sb, 0, 0)
        x_chunk_process(xt_chunks[2], xTt_all, 2, "xct2")
        x_chunk_process(xt_chunks[3], xTt_all, 3, "xct3")
        proj_half(KT0, wk_all, xTt_all, bk_sb, 0, 1)

        scores_jt(QT0, KT0, 0, Et0)
        scores_jt(QT0, KT0, 1, Et0)
        scores_jt(QT0, KT0, 2, Et0)
        proj_half(VT0, wv_all, xTt_all, bv_sb, 0, 0)
        scores_jt(QT0, KT0, 3, Et0)
        proj_half(VT0, wv_all, xTt_all, bv_sb, 0, 1)
        scores_jt(QT0, KT0, 4, Et0)
        vprime_strips(VT0, Vp0, range(0, 4))
        scores_jt(QT0, KT0, 5, Et0)
        vprime_strips(VT0, Vp0, range(4, 8))
        scores_jt(QT0, KT0, 6, Et0)
        vprime_ones(Vp0)
        scores_jt(QT0, KT0, 7, Et0)

        prev = {"mt": 0, "Vp": Vp0, "Et": Et0}
        prev.update(new_back_state(0))

        # ---- steady pairs 1..7: front(p) interleaved with back(p-1) ----
        for hp in range(1, NP):
            mt = hp
            Et = {}
            QTp = pp_pool.tile([P, S], BF16, tag="qt", bufs=2, name="QTp")
            KTp = pp_pool.tile([P, S], BF16, tag="kt", bufs=2, name="KTp")
            VTp = pp_pool.tile([P, S], BF16, tag="vt", bufs=2, name="VTp")
            Vp = pp_pool.tile([P, ST * VW], BF16, tag="vp", bufs=2, name="Vp")

            proj_half(QTp, wq_all, xTf_all, bq_sb, mt, 0)
            ctx_chunk(prev, 0, 0)
            proj_half(QTp, wq_all, xTf_all, bq_sb, mt, 1)
            ctx_chunk(prev, 0, 1)
            proj_half(KTp, wk_all, xTt_all, bk_sb, mt, 0)
            outT_norm(prev, 0)
            proj_half(KTp, wk_all, xTt_all, bk_sb, mt, 1)
            ctx_chunk(prev, 1, 0)
            scores_jt(QTp, KTp, 0, Et)
            ctx_chunk(prev, 1, 1)
            scores_jt(QTp, KTp, 1, Et)
            outT_norm(prev, 1)
            scores_jt(QTp, KTp, 2, Et)
            out_dma(prev)
            scores_jt(QTp, KTp, 3, Et)
            proj_half(VTp, wv_all, xTt_all, bv_sb, mt, 0)
            scores_jt(QTp, KTp, 4, Et)
            proj_half(VTp, wv_all, xTt_all, bv_sb, mt, 1)
            scores_jt(QTp, KTp, 5, Et)
            vprime_strips(VTp, Vp, range(0, 4))
            scores_jt(QTp, KTp, 6, Et)
            vprime_strips(VTp, Vp, range(4, 8))
            vprime_ones(Vp)
            scores_jt(QTp, KTp, 7, Et)

            prev = {"mt": mt, "Vp": Vp, "Et": Et}
            prev.update(new_back_state(mt))

        # ---- drain: back(7) ----
        ctx_chunk(prev, 0, 0)
        ctx_chunk(prev, 0, 1)
        outT_norm(prev, 0)
        ctx_chunk(prev, 1, 0)
        ctx_chunk(prev, 1, 1)
        outT_norm(prev, 1)
        out_dma(prev)

    nc.compile()
    return nc


def run(inputs, trace=False, trace_kwargs=None):
    """inputs: dict of full-shape np arrays as in reference.setup_inputs()."""
    nc = build_kernel()
    in_maps = []
    for b in range(N_CORES):
        in_maps.append({
            "from_tensor": np.ascontiguousarray(np.asarray(inputs["from_tensor"][b], dtype=np.float32)),
            "to_tensor": np.ascontiguousarray(np.asarray(inputs["to_tensor"][b], dtype=np.float32)),
            "Wq": np.asarray(inputs["Wq"], dtype=np.float32),
            "bq": np.asarray(inputs["bq"], dtype=np.float32),
            "Wk": np.asarray(inputs["Wk"], dtype=np.float32),
            "bk": np.asarray(inputs["bk"], dtype=np.float32),
            "Wv": np.asarray(inputs["Wv"], dtype=np.float32),
            "bv": np.asarray(inputs["bv"], dtype=np.float32),
        })
    res = run_bass_kernel_spmd(nc, in_maps, core_ids=list(range(N_CORES)),
                               trace=trace, **(trace_kwargs or {}))
    out = np.stack([np.asarray(res.results[b]["out"]) for b in range(N_CORES)],
                   axis=0).astype(np.float32)
    return out, res


def kernel(**inputs):
    out, _ = run(inputs, trace=False)
    return out


# revision 6
# speedup vs baseline: 1.1067x; 1.0608x over previous
"""Multi-head attention forward (B=8, S=1024, H=16, D=64) on 8 TRN2 NeuronCores.

Sharding: pure data-parallel over batch — core b computes batch element b
end-to-end (QKV projections + 16-head attention). Zero collectives.

Per-core dataflow (bf16 matmuls, fp32 PSUM accumulation), scheduled around
DMA arrival so the PE and ScalarE pipelines start as early as possible:
  - weights for pairs 0-1 load as fine column slices (512B rows) first, so
    the first exp only waits on x + 1MB of weights; the remaining columns
    stream as per-kt row-block slices (3KB rows) while the pair loop runs.
  - x loads in 256-row chunks; transposes interleave with pair-0 projection
    chains; Q/K/V^T tiles are split into 512-column halves so scores can
    start after the lo half of x_to has landed.
  - biases load as [8,128] rows + one PE transpose each.
  - the pair loop software-pipelines back(p-1) work (ctx, out-transpose,
    normalize, store) into the exp-cadence gaps of front(p), with the next
    pair's Q/K chains as tail filler, so the PE stays dense at the pair
    boundary and ctx never waits on the previous pair's last exp.
  - V' strips build with one [128,128] PE transpose per s-tile (both heads);
    softmax denominators ride the ones-column of V' and are reciprocal'd
    8-at-a-time after the output transpose.
"""

import numpy as np
from contextlib import ExitStack

import concourse.bass as bass
import concourse.mybir as mybir
import concourse.tile as tile
from concourse import bacc
from concourse.masks import make_identity
from concourse.bass_utils import run_bass_kernel_spmd

B, S, H, D = 8, 1024, 16, 64
W = H * D  # 1024
P = 128
N_CORES = 8
F32 = mybir.dt.float32
BF16 = mybir.dt.bfloat16
AF = mybir.ActivationFunctionType
ALU = mybir.AluOpType

ST = S // P   # 8 s-tiles
KT_ = W // P  # 8 contraction tiles
IH = 2        # 512-wide halves of the moving dim
HD1 = D + 1   # 65: V' width per head
NP = H // 2   # 8 head pairs
VW = 2 * HD1  # 130: V' slot width per s-tile (two heads + ones cols)
NFINE = 2     # pairs covered by fine column-sliced weight loads
BW = W - NFINE * P  # 768: bulk column width


def _dedup_ldweights(nc):
    removed = 0
    for f in nc.m.functions:
        for blk in f.blocks:
            ins = blk.instructions
            last_key = None
            to_remove = []
            for i in ins:
                if str(getattr(i, "engine", None)) != "EngineType.PE":
                    continue
                tn = type(i).__name__
                if tn == "InstLdweights":
                    si = i.sync_info
                    clean = si is None or (not si.on_wait and not si.on_update)
                    key = (str(i.ins), str(getattr(i, "is_transpose", None)),
                           str(getattr(i, "tile_position", None)),
                           str(getattr(i, "perf_mode", None)))
                    if clean and key == last_key:
                        to_remove.append(i)
                    else:
                        last_key = key
                elif tn != "InstMatmult":
                    last_key = None
            for i in to_remove:
                ins.remove(i)
            removed += len(to_remove)
    return removed


def build_kernel():
    nc = bacc.Bacc(trn_type="TRN2", target_bir_lowering=False, debug=False,
                   num_devices=N_CORES)

    xf_ext = nc.dram_tensor("from_tensor", [S, W], F32, kind="ExternalInput").ap()
    xt_ext = nc.dram_tensor("to_tensor", [S, W], F32, kind="ExternalInput").ap()
    wq_ext = nc.dram_tensor("Wq", [W, W], F32, kind="ExternalInput").ap()
    bq_ext = nc.dram_tensor("bq", [W], F32, kind="ExternalInput").ap()
    wk_ext = nc.dram_tensor("Wk", [W, W], F32, kind="ExternalInput").ap()
    bk_ext = nc.dram_tensor("bk", [W], F32, kind="ExternalInput").ap()
    wv_ext = nc.dram_tensor("Wv", [W, W], F32, kind="ExternalInput").ap()
    bv_ext = nc.dram_tensor("bv", [W], F32, kind="ExternalInput").ap()
    out_ext = nc.dram_tensor("out", [S, W], F32, kind="ExternalOutput").ap()

    with tile.TileContext(nc) as tc, ExitStack() as top:
        const = top.enter_context(tc.tile_pool(name="const", bufs=1))
        big = top.enter_context(tc.tile_pool(name="big", bufs=1))
        xr_pool = top.enter_context(tc.tile_pool(name="xr", bufs=2))
        xc_pool = top.enter_context(tc.tile_pool(name="xc", bufs=2))
        pp_pool = top.enter_context(tc.tile_pool(name="pp", bufs=1))
        et_pool = top.enter_context(tc.tile_pool(name="et", bufs=17))
        sm_pool = top.enter_context(tc.tile_pool(name="sm", bufs=1))
        ps_proj = top.enter_context(
            tc.tile_pool(name="ps_proj", bufs=2, space="PSUM"))
        ps_s = top.enter_context(
            tc.tile_pool(name="ps_s", bufs=1, space="PSUM"))
        ps_c = top.enter_context(
            tc.tile_pool(name="ps_c", bufs=2, space="PSUM"))

        # ---- identity matrices (gpsimd queue, before weight descriptors) ----
        ident = const.tile([P, P], BF16, tag="ident")
        make_identity(nc, ident[:])
        idf32 = const.tile([8, 8], F32, tag="idf32")
        make_identity(nc, idf32[:])

        # ---- DMA issue: biases + x chunks on sync (HWDGE) queue ----
        brow = const.tile([8, 3 * P], F32, tag="brow")
        for i, b_ext in enumerate((bq_ext, bk_ext, bv_ext)):
            nc.sync.dma_start(brow[:, i * P:(i + 1) * P],
                              b_ext.rearrange("(t p) -> t p", p=P))

        xTf = [big.tile([P, KT_ * 512], BF16, tag=f"xTf{h}", name=f"xTf{h}") for h in range(2)]
        xTt = [big.tile([P, KT_ * 512], BF16, tag=f"xTt{h}", name=f"xTt{h}") for h in range(2)]

        def x_chunk_load(x_ext, ch, name):
            xr = xr_pool.tile([P, 2 * W], F32, tag="xr", name=name)
            nc.sync.dma_start(
                xr.rearrange("p (t f) -> p t f", f=W),
                x_ext.rearrange("(t p) f -> p t f", p=P)[
                    :, ch * 2:(ch + 1) * 2, :])
            return xr

        # ---- weight loads (gpsimd / SWDGE queue) ----
        # fine column slices for pairs 0..NFINE-1 first (tiny rows, land in a
        # few us); then per-kt row-block slices of the remaining columns.
        wfine = {}
        for nm, ext in (("q", wq_ext), ("k", wk_ext), ("v", wv_ext)):
            for mt in range(NFINE):
                wfine[nm, mt] = big.tile([P, KT_ * P], BF16,
                                         tag=f"wf{nm}{mt}", name=f"wf{nm}{mt}")
        wbulk = {}
        for nm, ext in (("q", wq_ext), ("k", wk_ext), ("v", wv_ext)):
            for kt in range(KT_):
                wbulk[nm, kt] = big.tile([P, BW], BF16, tag=f"wb{nm}{kt}", name=f"wb{nm}{kt}")

        def load_w_fine(nm, ext, mt):
            nc.gpsimd.dma_start(
                wfine[nm, mt].rearrange("p (t c) -> p t c", c=P),
                ext.rearrange("(t p) f -> p t f", p=P)[
                    :, :, mt * P:(mt + 1) * P])

        def load_w_bulk(nm, ext, kt):
            nc.gpsimd.dma_start(
                wbulk[nm, kt][:],
                ext.rearrange("(t p) f -> p t f", p=P)[
                    :, kt, NFINE * P:])

        load_w_fine("q", wq_ext, 0)
        load_w_fine("k", wk_ext, 0)
        load_w_fine("q", wq_ext, 1)
        load_w_fine("k", wk_ext, 1)
        load_w_fine("v", wv_ext, 0)
        load_w_fine("v", wv_ext, 1)
        for nm, ext in (("q", wq_ext), ("k", wk_ext), ("v", wv_ext)):
            for kt in range(KT_):
                load_w_bulk(nm, ext, kt)

        def w_slice(nm, mt, kt):
            if mt < NFINE:
                return wfine[nm, mt][:, kt * P:(kt + 1) * P]
            return wbulk[nm, kt][:, (mt - NFINE) * P:(mt - NFINE + 1) * P]

        # ---- bias transpose: [8,128] rows -> [128,8] columns ----
        b_sb = const.tile([P, 24], F32, tag="b_sb")
        bps = ps_proj.tile([P, 24], F32, tag="proj", name="bps")
        for i in range(3):
            nc.tensor.transpose(bps[:, i * 8:(i + 1) * 8],
                                brow[:, i * P:(i + 1) * P], idf32[:])
        nc.vector.tensor_copy(b_sb[:], bps[:])
        b_of = {"q": 0, "k": 8, "v": 16}

        # ---- x chunk processing: cast + 16 transposes + 2 batched copies ----
        def x_chunk_process(xr, xT_half, sub, name):
            # sub: 0 or 1 = position of this 256-row chunk within the half
            xc = xc_pool.tile([P, 2 * W], BF16, tag="xc", name=name)
            nc.vector.tensor_copy(xc[:], xr[:])
            for sl in range(2):
                pt = ps_proj.tile([P, KT_ * P], BF16, tag="proj", name="ptx")
                for wt in range(KT_):
                    nc.tensor.transpose(
                        pt[:, wt * P:(wt + 1) * P],
                        xc[:, sl * W + wt * P: sl * W + wt * P + P],
                        ident[:])
                nc.vector.tensor_copy(
                    xT_half.rearrange("p (w s) -> p w s", s=512)[
                        :, :, sub * 256 + sl * P: sub * 256 + (sl + 1) * P],
                    pt.rearrange("p (w c) -> p w c", c=P))

        # ---- pair-loop building blocks ----
        def proj_half(dst_half, nm, xT, b_sl, mt, ih):
            ps = ps_proj.tile([P, 512], F32, tag="proj", name="pp")
            for kt in range(KT_):
                nc.tensor.matmul(
                    ps[:],
                    lhsT=w_slice(nm, mt, kt),
                    rhs=xT[ih][:, kt * 512:(kt + 1) * 512],
                    start=(kt == 0), stop=(kt == KT_ - 1))
            nc.vector.tensor_scalar_add(
                dst_half[:], ps[:], b_sl[:, mt:mt + 1])

        def scores_jt(QT, KT2, jt, Et):
            # both heads share ONE 4-bank PSUM tile; K=64 matmuls pack onto
            # disjoint PE row-groups and run concurrently.
            pss = ps_s.tile([P, 2 * S], F32, tag="pss", name="pss")
            kh = KT2[jt // 4]
            for ih in range(IH):
                for hh in range(2):
                    ho = hh * D
                    nc.tensor.matmul(
                        pss[:, hh * S + ih * 512: hh * S + (ih + 1) * 512],
                        lhsT=kh[ho:ho + D, (jt % 4) * P:(jt % 4) * P + P],
                        rhs=QT[ih][ho:ho + D, :],
                        start=True, stop=True)
            et = et_pool.tile([P, 2 * S], BF16, tag="et", name="et")
            nc.scalar.activation(et[:], pss[:], AF.Exp, scale=0.125)
            Et[jt] = et

        def vprime_strips(VT2, Vp, jts):
            for jt in jts:
                pv = ps_proj.tile([P, P], BF16, tag="proj", name="pv")
                nc.tensor.transpose(
                    pv[:], VT2[jt // 4][:, (jt % 4) * P:(jt % 4 + 1) * P],
                    ident[:])
                nc.vector.tensor_copy(
                    Vp.rearrange("p (j g c) -> p j g c", g=2, c=HD1)[
                        :, jt, :, 0:D],
                    pv.rearrange("p (g c) -> p g c", c=D))

        def vprime_ones(Vp):
            nc.vector.memset(
                Vp.rearrange("p (j g c) -> p j g c", g=2, c=HD1)[
                    :, :, :, D:HD1], 1.0)

        def ctx_chunk(prev, hh, ih):
            Vp, Et = prev["Vp"], prev["Et"]
            pc = ps_c.tile([HD1, 512], F32, tag="pc", name="pc")
            for jt in range(ST):
                nc.tensor.matmul(
                    pc[:],
                    lhsT=Vp[:, jt * VW + hh * HD1: jt * VW + (hh + 1) * HD1],
                    rhs=Et[jt][:, hh * S + ih * 512: hh * S + (ih + 1) * 512],
                    start=(jt == 0), stop=(jt == ST - 1))
            nc.vector.tensor_copy(
                prev["ctxb"][hh][:, ih * 512:(ih + 1) * 512], pc[:])

        def outT_norm(prev, hh):
            ctxb = prev["ctxb"][hh]
            out_p = prev["out_p"]
            # stride 66 keeps each bf16 PSUM slice 4-byte aligned
            po = ps_proj.tile([P, ST * 66], BF16, tag="proj", name="po")
            for it in range(ST):
                nc.tensor.transpose(
                    po[:, it * 66: it * 66 + HD1],
                    ctxb[:, it * P:(it + 1) * P],
                    ident[0:HD1, 0:HD1])
            rinv = sm_pool.tile([P, 8], F32, tag="rinv", bufs=3, name="rinv")
            nc.vector.reciprocal(
                rinv.rearrange("p (a b) -> p a b", b=1),
                po.rearrange("p (it c) -> p it c", c=66)[:, :, D:HD1])
            for it in range(ST):
                nc.vector.tensor_scalar_mul(
                    out_p[:, it * P + hh * D: it * P + hh * D + D],
                    po[:, it * 66: it * 66 + D], rinv[:, it:it + 1])

        def out_dma(prev):
            nc.sync.dma_start(
                out_ext.rearrange("(t p) (g c) -> p t g c", p=P, c=P)[
                    :, :, prev["mt"], :],
                prev["out_p"].rearrange("p (t c) -> p t c", c=P))

        def pair_tiles(mt):
            QT = [pp_pool.tile([P, 512], BF16, tag=f"qt{h}", bufs=2,
                               name="QT") for h in range(2)]
            KT2 = [pp_pool.tile([P, 512], BF16, tag=f"kt{h}", bufs=2,
                                name="KT") for h in range(2)]
            VT2 = [pp_pool.tile([P, 512], BF16, tag=f"vt{h}", bufs=2,
                                name="VT") for h in range(2)]
            Vp = pp_pool.tile([P, ST * VW], BF16, tag="vp", bufs=2, name="Vp")
            ctxb0 = sm_pool.tile([HD1, S], BF16, tag="ctxb", bufs=3,
                                 name="ctxb0")
            ctxb1 = sm_pool.tile([HD1, S], BF16, tag="ctxb", bufs=3,
                                 name="ctxb1")
            out_p = pp_pool.tile([P, ST * P], F32, tag="outp", bufs=2,
                                 name="out_p")
            return {"mt": mt, "QT": QT, "KT2": KT2, "VT2": VT2, "Vp": Vp,
                    "ctxb": (ctxb0, ctxb1), "out_p": out_p, "Et": {}}

        # ---- pair 0 + phase 0, interleaved ----
        p0 = pair_tiles(0)
        xf_chunks = [x_chunk_load(xf_ext, ch, f"xrf{ch}") for ch in range(2)]
        x_chunk_process(xf_chunks[0], xTf[0], 0, "xcf0")
        xf_chunks.append(x_chunk_load(xf_ext, 2, "xrf2"))
        x_chunk_process(xf_chunks[1], xTf[0], 1, "xcf1")
        xf_chunks.append(x_chunk_load(xf_ext, 3, "xrf3"))
        proj_half(p0["QT"][0], "q", xTf, b_sb[:, 0:8], 0, 0)
        x_chunk_process(xf_chunks[2], xTf[1], 0, "xcf2")
        xt_chunks = [x_chunk_load(xt_ext, 0, "xrt0")]
        x_chunk_process(xf_chunks[3], xTf[1], 1, "xcf3")
        xt_chunks.append(x_chunk_load(xt_ext, 1, "xrt1"))
        proj_half(p0["QT"][1], "q", xTf, b_sb[:, 0:8], 0, 1)
        x_chunk_process(xt_chunks[0], xTt[0], 0, "xct0")
        xt_chunks.append(x_chunk_load(xt_ext, 2, "xrt2"))
        x_chunk_process(xt_chunks[1], xTt[0], 1, "xct1")
        xt_chunks.append(x_chunk_load(xt_ext, 3, "xrt3"))
        proj_half(p0["KT2"][0], "k", xTt, b_sb[:, 8:16], 0, 0)
        scores_jt(p0["QT"], p0["KT2"], 0, p0["Et"])
        x_chunk_process(xt_chunks[2], xTt[1], 0, "xct2")
        scores_jt(p0["QT"], p0["KT2"], 1, p0["Et"])
        x_chunk_process(xt_chunks[3], xTt[1], 1, "xct3")
        scores_jt(p0["QT"], p0["KT2"], 2, p0["Et"])
        proj_half(p0["KT2"][1], "k", xTt, b_sb[:, 8:16], 0, 1)
        scores_jt(p0["QT"], p0["KT2"], 3, p0["Et"])
        proj_half(p0["VT2"][0], "v", xTt, b_sb[:, 16:24], 0, 0)
        scores_jt(p0["QT"], p0["KT2"], 4, p0["Et"])
        proj_half(p0["VT2"][1], "v", xTt, b_sb[:, 16:24], 0, 1)
        scores_jt(p0["QT"], p0["KT2"], 5, p0["Et"])
        vprime_strips(p0["VT2"], p0["Vp"], range(0, 4))
        scores_jt(p0["QT"], p0["KT2"], 6, p0["Et"])
        vprime_strips(p0["VT2"], p0["Vp"], range(4, 8))
        vprime_ones(p0["Vp"])
        scores_jt(p0["QT"], p0["KT2"], 7, p0["Et"])

        prev = p0

        # ---- steady pairs 1..7: front(p) interleaved with back(p-1) ----
        for hp in range(1, NP):
            mt = hp
            cur = pair_tiles(mt)
            bq_sl = b_sb[:, 0:8]
            bk_sl = b_sb[:, 8:16]
            bv_sl = b_sb[:, 16:24]

            proj_half(cur["QT"][0], "q", xTf, bq_sl, mt, 0)
            proj_half(cur["QT"][1], "q", xTf, bq_sl, mt, 1)
            ctx_chunk(prev, 0, 0)
            proj_half(cur["KT2"][0], "k", xTt, bk_sl, mt, 0)
            proj_half(cur["KT2"][1], "k", xTt, bk_sl, mt, 1)
            scores_jt(cur["QT"], cur["KT2"], 0, cur["Et"])
            ctx_chunk(prev, 0, 1)
            scores_jt(cur["QT"], cur["KT2"], 1, cur["Et"])
            outT_norm(prev, 0)
            scores_jt(cur["QT"], cur["KT2"], 2, cur["Et"])
            ctx_chunk(prev, 1, 0)
            scores_jt(cur["QT"], cur["KT2"], 3, cur["Et"])
            ctx_chunk(prev, 1, 1)
            scores_jt(cur["QT"], cur["KT2"], 4, cur["Et"])
            outT_norm(prev, 1)
            scores_jt(cur["QT"], cur["KT2"], 5, cur["Et"])
            proj_half(cur["VT2"][0], "v", xTt, bv_sl, mt, 0)
            scores_jt(cur["QT"], cur["KT2"], 6, cur["Et"])
            proj_half(cur["VT2"][1], "v", xTt, bv_sl, mt, 1)
            vprime_strips(cur["VT2"], cur["Vp"], range(0, 4))
            scores_jt(cur["QT"], cur["KT2"], 7, cur["Et"])
            vprime_strips(cur["VT2"], cur["Vp"], range(4, 8))
            vprime_ones(cur["Vp"])
            out_dma(prev)

            prev = cur

        # ---- drain: back(7) ----
        ctx_chunk(prev, 0, 0)
        ctx_chunk(prev, 0, 1)
        outT_norm(prev, 0)
        ctx_chunk(prev, 1, 0)
        ctx_chunk(prev, 1, 1)
        outT_norm(prev, 1)
        out_dma(prev)

    nc.compile()
    return nc


def run(inputs, trace=False, trace_kwargs=None):
    """inputs: dict of full-shape np arrays as in reference.setup_inputs()."""
    nc = build_kernel()
    in_maps = []
    for b in range(N_CORES):
        in_maps.append({
            "from_tensor": np.ascontiguousarray(np.asarray(inputs["from_tensor"][b], dtype=np.float32)),
            "to_tensor": np.ascontiguousarray(np.asarray(inputs["to_tensor"][b], dtype=np.float32)),
            "Wq": np.asarray(inputs["Wq"], dtype=np.float32),
            "bq": np.asarray(inputs["bq"], dtype=np.float32),
            "Wk": np.asarray(inputs["Wk"], dtype=np.float32),
            "bk": np.asarray(inputs["bk"], dtype=np.float32),
            "Wv": np.asarray(inputs["Wv"], dtype=np.float32),
            "bv": np.asarray(inputs["bv"], dtype=np.float32),
        })
    res = run_bass_kernel_spmd(nc, in_maps, core_ids=list(range(N_CORES)),
                               trace=trace, **(trace_kwargs or {}))
    out = np.stack([np.asarray(res.results[b]["out"]) for b in range(N_CORES)],
                   axis=0).astype(np.float32)
    return out, res


def kernel(**inputs):
    out, _ = run(inputs, trace=False)
    return out
